# revision 48
# baseline (speedup 1.0000x reference)
"""Trainium2 Bass kernel for nn_Block_ToMeATC (B=8,N=1024,C=768,H=12, token_ratio=0.5).

Strategy: data-parallel over batch (1 row per NeuronCore, 8 cores).
Per core: LN1 -> qkv (LN affine folded into weights on host) -> size-biased
attention in S^T layout (softmax denominator folded into the P@V matmul as a
ones column) -> proj + residual -> ToMe bipartite merge.  Since
r = N - int(0.5*N) = N/2, ALL even tokens merge, so the argsort in the
reference is irrelevant; only the per-row argmax of the cosine-similarity
scores matters, implemented as a one-hot (score == rowmax) matrix and a
matmul-based scatter-add.  Then LN2 -> MLP (gelu) -> residual.

Numerics: metric/scores path in fp32 (argmax top-2 gap is ~5e-6), all large
GEMMs in bf16 (measured end-to-end rel err ~2e-3).  Softmax runs without
max-subtraction (max |logit| ~ 8.8, verified safe).
"""
import sys

if "/opt/trn_rl_repo" not in sys.path:
    sys.path.insert(0, "/opt/trn_rl_repo")

import numpy as np
import ml_dtypes

import concourse.bass as bass
import concourse.mybir as mybir
import concourse.tile as tile
from concourse.masks import make_identity
from concourse.vector_clock import ScopedClock, VectorClock

F32 = mybir.dt.float32
BF16 = mybir.dt.bfloat16
AF = mybir.ActivationFunctionType
OP = mybir.AluOpType

P = 128
N_TOK, C, H, HD = 1024, 768, 12, 64
NH = N_TOK // 2            # 512 (even tokens / odd tokens / output tokens)
KC = C // P                # 6 K-tiles over channels
NT = N_TOK // P            # 8 token tiles
EPS = 1e-5
N_CORES = 8


# ---------------------------------------------------------------------------
# Workaround for walrus "Too many sync wait commands": the walrus build in this
# container rejects instructions carrying more than MAX_WAITS semaphore waits.
# Post-pass: move excess waits onto same-engine NoOps inserted just before the
# instruction.  Also split the TileContext tail drain the same way.
import bass_rust as _bass_rust

MAX_WAITS = 1


def _split_excess_waits(nc):
    for f in nc.m.functions:
        for bb in f.blocks:
            new_list = []
            changed = False
            for ins in bb.instructions:
                si = getattr(ins, "sync_info", None)
                w = list(si.on_wait) if si is not None and si.on_wait else []
                if len(w) > MAX_WAITS:
                    changed = True
                    k = 0
                    while len(w) > MAX_WAITS:
                        chunk, w = w[:MAX_WAITS], w[MAX_WAITS:]
                        nop = mybir.InstNoOp(
                            name=f"{ins.name}-ws{k}", ins=[], outs=[])
                        nop.engine = ins.engine
                        nop.sync_info = _bass_rust.SyncInfo(
                            on_wait=chunk, on_update=[])
                        nc.register_instruction(nop, overwrite=True)
                        new_list.append(nop)
                        k += 1
                    ins.sync_info = _bass_rust.SyncInfo(
                        on_wait=w, on_update=list(si.on_update or []))
                new_list.append(ins)
            if changed:
                bb.instructions[:] = new_list


def _split_drain_and_barrier(self, tick_clock, wait_clock):
    vc = tick_clock.global_clock
    n = len(vc)
    for i in range(n):
        t = vc[i]
        if t > 0:
            part = VectorClock([t if j == i else 0 for j in range(n)])
            nop_inst = self.nc.sync.nop(nofuse=True)
            wait_clock.add_sem_waits(nop_inst.ins, ScopedClock({None: part}))
    drain_inst = self.nc.sync.drain()
    wait_clock.add_sem_waits(
        drain_inst.ins, ScopedClock({None: vc.copy()}), ScopedClock({None: vc.copy()})
    )
    self.nc.all_engine_barrier()
    assert self.sems is not None
    popped = self.nc._tile_sem_poison_stack.pop()
    assert popped is self._sem_poison
    self.nc.clear_and_free_semaphores(list(self.sems.allocated().values()))
    self.nc.all_engine_barrier()
    _split_excess_waits(self.nc)


tile.TileContext._drain_and_barrier = _split_drain_and_barrier
# ---------------------------------------------------------------------------


def _bcast_row(nc, out_ap, dram_row_ap, nparts):
    """DMA-broadcast a [1, X] DRAM row across `nparts` partitions."""
    src = bass.AP(
        tensor=dram_row_ap.tensor,
        offset=dram_row_ap.offset,
        ap=[[0, nparts]] + dram_row_ap.ap[1:],
    )
    nc.sync.dma_start(out=out_ap, in_=src)


def _layernorm_tile(nc, pool, x_ap, z_out_ap, eps_tile, apply_engine=None):
    """z = (x - mean(x)) * rsqrt(var(x) + eps) over the free dim (768)."""
    sub = 256  # gcd(512, 768)
    nsub = C // sub
    stats = pool.tile([P, nsub, 6], F32, tag="stats")
    xin = x_ap.rearrange("p (s d) -> p s d", s=nsub)
    for s in range(nsub):
        nc.vector.bn_stats(out=stats[:, s, :], in_=xin[:, s, :])
    mv = pool.tile([P, 2], F32, tag="mv")
    nc.vector.bn_aggr(out=mv[:], in_=stats[:])
    rstd = pool.tile([P, 1], F32, tag="rstd")
    nc.scalar.activation(out=rstd[:], in_=mv[:, 1:2], func=AF.Sqrt,
                         bias=eps_tile[:], scale=1.0)
    nc.vector.reciprocal(out=rstd[:], in_=rstd[:])
    eng = apply_engine if apply_engine is not None else nc.vector
    eng.tensor_scalar(out=z_out_ap, in0=x_ap, scalar1=mv[:, 0:1],
                      scalar2=rstd[:], op0=OP.subtract, op1=OP.mult)


def _build_nc(unit_size):
    nc = bass.Bass()

    # --- DRAM I/O (per-core slices; weights are identical on every core) ---
    xp = nc.dram_tensor("xp", [N_TOK, C], F32, kind="ExternalInput")
    wqkT = nc.dram_tensor("wqkT", [C, 2 * C], BF16, kind="ExternalInput")
    wvT = nc.dram_tensor("wvT", [C, C], BF16, kind="ExternalInput")
    wmT2 = nc.dram_tensor("wmT2", [P, KC * HD], F32, kind="ExternalInput")
    projT = nc.dram_tensor("projT", [C, C], BF16, kind="ExternalInput")
    fc1T = nc.dram_tensor("fc1T", [C, 4 * C], BF16, kind="ExternalInput")
    fc2T = nc.dram_tensor("fc2T", [4 * C, C], BF16, kind="ExternalInput")
    # packed per-partition constants: bqk(12) | b3(24) | bm(1) | asz(8) | lsz(8)
    cpack = nc.dram_tensor("cpack", [P, 53], F32, kind="ExternalInput")
    # packed broadcast rows: bv | proj_b | fc2_b
    rpack = nc.dram_tensor("rpack", [1, 3 * C], F32, kind="ExternalInput")

    out_x = nc.dram_tensor("out_x", [NH, C], F32, kind="ExternalOutput")
    out_sz = nc.dram_tensor("out_sz", [NH, 1], F32, kind="ExternalOutput")

    rin_scr = nc.dram_tensor("rin_scr", [1, NH], F32)      # internal scratch
    den_scr = nc.dram_tensor("den_scr", [H, N_TOK], F32)   # internal scratch

    with tile.TileContext(nc) as tc:
        # ---- long-lived pools (L0) ----
        cm_const = tc.tile_pool(name="const", bufs=1); const = cm_const.__enter__()
        cm_scr = tc.tile_pool(name="scr", bufs=3); scratch = cm_scr.__enter__()
        cm_stat = tc.tile_pool(name="stat", bufs=8); stat = cm_stat.__enter__()

        ident = const.tile([P, P], F32)
        make_identity(nc, ident[:])
        eps_t = const.tile([P, 1], F32)
        nc.vector.memset(eps_t[:], EPS)
        ones64 = const.tile([P, 1], F32)
        nc.vector.memset(ones64[:], 1.0)
        ones_row = const.tile([1, HD], F32)
        nc.vector.memset(ones_row[:], 1.0)
        cpack_sb = const.tile([P, 53], F32)
        nc.gpsimd.dma_start(out=cpack_sb[:], in_=cpack[:, :])
        bqk_sb = cpack_sb[:, 0:12]
        b3_sb = cpack_sb[:, 12:36]
        bm_sb = cpack_sb[:, 36:37]
        asz_sb = cpack_sb[:, 37:45]
        lsz_sb = cpack_sb[:, 45:53]
        rows_b = const.tile([P, 3 * C], F32)
        _bcast_row(nc, rows_b[:], rpack[:, :], P)
        bv_b = rows_b[:, 0:C]
        pb_b = rows_b[:, C:2 * C]
        f2b_b = rows_b[:, 2 * C:3 * C]
        wm_sb = const.tile([P, KC, HD], F32)
        nc.gpsimd.dma_start(out=wm_sb[:], in_=wmT2[:, :])
        metricT = const.tile([P, N_TOK], F32)

        # ---- lifetime-scoped pools, opened in descending close-time order ----
        cm_oh = tc.tile_pool(name="p_oh", bufs=1, side="right")    # closes end-F
        onehot = cm_oh.__enter__().tile([P, 4, NH], F32)
        cm_x1w = tc.tile_pool(name="p_x1w", bufs=1, side="right")  # closes end-F
        x1w = cm_x1w.__enter__().tile([P, NT, C + 1], F32)
        cm_xs = tc.tile_pool(name="p_xs", bufs=1)          # closes end-E
        xs = cm_xs.__enter__().tile([P, NT, C], F32)       # x (+proj_b), A..E
        cm_xaT = tc.tile_pool(name="p_xaT", bufs=1)        # closes end-E
        _xaT_pool = cm_xaT.__enter__()
        # one tile per head-pair so proj can start as soon as a pair finishes
        xaT16 = [_xaT_pool.tile([P, N_TOK], BF16, tag=f"xa{i}", name=f"xa{i}")
                 for i in range(KC)]
        cm_qkT = tc.tile_pool(name="p_qkT", bufs=1)        # closes end-D
        _qkT_pool = cm_qkT.__enter__()
        # one tile per 128-outc block so attention pair p can start once
        # blocks p and 6+p are done (overlaps with the rest of phase B)
        qkT = [_qkT_pool.tile([P, N_TOK], BF16, tag=f"qk{i}", name=f"qk{i}")
               for i in range(12)]
        cm_vaug = tc.tile_pool(name="p_vaug", bufs=1)      # closes end-D
        _vaug_pool = cm_vaug.__enter__()
        vaug = [_vaug_pool.tile([P, H, HD + 1], BF16, tag=f"va{i}", name=f"va{i}")
                for i in range(NT)]
        cm_zt16 = tc.tile_pool(name="p_zt16", bufs=1)      # closes end-B
        zt16 = cm_zt16.__enter__().tile([P, KC, N_TOK], BF16)

        # =========== Phase A: load x, LN1, transpose z^T, metric GEMM ===========
        with tc.tile_pool(name="psA", bufs=3, space="PSUM") as psA:
            psmet = psA.tile([P, N_TOK], F32, tag="met", bufs=1)
            for t in range(NT):
                nc.sync.dma_start(out=xs[0:64, t, :],
                                  in_=xp[t * P:t * P + 64, :])
                nc.gpsimd.dma_start(out=xs[64:128, t, :],
                                    in_=xp[t * P + 64:(t + 1) * P, :])
                z = scratch.tile([P, C], F32, tag="z")
                _layernorm_tile(nc, stat, xs[:, t, :], z[:], eps_t)
                # all 6 transposed blocks land in one 2-bank psum tile, so the
                # fp32 and bf16 copies are one wide op each instead of 6 narrow
                pt = psA.tile([P, C], F32, tag="tr", bufs=3)
                for kc in range(KC):
                    nc.tensor.transpose(out=pt[:, kc * P:(kc + 1) * P],
                                        in_=z[:, kc * P:(kc + 1) * P],
                                        identity=ident[:])
                zt32s = scratch.tile([P, KC, P], F32, tag="zt32s")
                nc.vector.tensor_copy(out=zt32s[:], in_=pt[:])
                nc.scalar.copy(
                    out=zt16[:, :, t * P:(t + 1) * P],
                    in_=pt[:].rearrange("p (a b) -> p a b", a=KC))
                tb = slice(t * P, (t + 1) * P)
                for kc in range(KC):
                    nc.tensor.matmul(out=psmet[0:HD, tb], lhsT=wm_sb[:, kc, :],
                                     rhs=zt32s[:, kc, :],
                                     start=(kc == 0), stop=(kc == KC - 1))
            nc.vector.tensor_scalar_add(out=metricT[0:HD, :], in0=psmet[0:HD, :],
                                        scalar1=bm_sb[0:HD, :])

        # ===== Phase C1: normalize odd metric columns (latency hides under B) =====
        with tc.tile_pool(name="psN", bufs=1, space="PSUM") as psN:
            sq = scratch.tile([P, NH], F32, tag="tt")
            nc.scalar.activation(out=sq[0:HD, :], in_=metricT[0:HD, NH:], func=AF.Square)
            psn = psN.tile([P, NH], F32, tag="psn")
            nc.tensor.matmul(out=psn[0:1, :], lhsT=ones64[0:HD, 0:1], rhs=sq[0:HD, :],
                             start=True, stop=True)
            rn = stat.tile([1, NH], F32, tag="rn", bufs=1)
            nc.scalar.activation(out=rn[:], in_=psn[0:1, :], func=AF.Sqrt)
            nc.vector.reciprocal(out=rn[:], in_=rn[:])
            nc.sync.dma_start(out=rin_scr[:, :], in_=rn[:])
            rinb = scratch.tile([P, NH], F32, tag="num")
            _bcast_row(nc, rinb[0:HD, :], rin_scr[:, :], HD)
            bhat = scratch.tile([P, NH], F32, tag="bhat", bufs=1)
            nc.vector.tensor_tensor(out=bhat[0:HD, :], in0=metricT[0:HD, NH:],
                                    in1=rinb[0:HD, :], op=OP.mult)

        # ====== Phase BD: v GEMM, then per-pair [attention(pr) ⊕ qk(pr+1)] ======
        # Software-pipelined: the qk GEMM for pair pr+1 is interleaved into the
        # ACT-bound attention loop of pair pr so the in-order PE always has
        # ready work while ScalarE computes exps.
        # PSUM budget: st 1x[128,1024] (2 banks) + pv 2x (4) + qk 1x (2) = 8.
        with (
            tc.tile_pool(name="wB", bufs=1) as wB,
            tc.tile_pool(name="psQK", bufs=4, space="PSUM") as psQK,
        ):
            wqk_sb = wB.tile([P, KC, 2 * C], BF16)
            wv_sb = wB.tile([P, KC, C], BF16)
            wengs = (nc.sync, nc.gpsimd)
            for kc in range(KC):
                wengs[kc % 2].dma_start(out=wv_sb[:, kc, :],
                                        in_=wvT[kc * P:(kc + 1) * P, :])
            for kc in range(KC):
                wengs[kc % 2].dma_start(out=wqk_sb[:, kc, :],
                                        in_=wqkT[kc * P:(kc + 1) * P, :])

            # fold proj_b into the residual copy of x (idle GPSIMD engine)
            for t in range(NT):
                nc.gpsimd.tensor_tensor(out=xs[:, t, :], in0=xs[:, t, :],
                                        in1=pb_b[:], op=OP.add)

            # v token-major: out[M=token(8 tiles), N=v_outc(768)] + ones col
            for t in range(NT):
                nc.vector.memset(vaug[t][:, :, HD:HD + 1], 1.0)
                ps = psQK.tile([P, N_TOK], F32, tag="qk")
                for (c0, c1) in ((0, 512), (512, 768)):
                    for kc in range(KC):
                        nc.tensor.matmul(
                            out=ps[:, c0:c1],
                            lhsT=zt16[:, kc, t * P:(t + 1) * P],
                            rhs=wv_sb[:, kc, c0:c1],
                            start=(kc == 0), stop=(kc == KC - 1),
                        )
                nc.vector.tensor_tensor(
                    out=vaug[t][:, :, 0:HD],
                    in0=ps[:, 0:C].rearrange("p (h d) -> p h d", h=H),
                    in1=bv_b[:].rearrange("p (h d) -> p h d", h=H),
                    op=OP.add,
                )

            # q^T,k^T: out[M=outc(12 tiles), N=token]; emit per head-pair so
            # attention pair p unblocks as early as possible
            for pr in range(6):
                for mo in (pr, 6 + pr):
                    ps = psQK.tile([P, N_TOK], F32, tag="qk")
                    for ch in range(2):
                        sl = slice(ch * 512, (ch + 1) * 512)
                        for kc in range(KC):
                            nc.tensor.matmul(
                                out=ps[:, sl],
                                lhsT=wqk_sb[:, kc, mo * P:(mo + 1) * P],
                                rhs=zt16[:, kc, sl],
                                start=(kc == 0), stop=(kc == KC - 1),
                            )
                    nc.scalar.activation(out=qkT[mo][:], in_=ps[:],
                                         func=AF.Identity,
                                         bias=bqk_sb[:, mo:mo + 1], scale=1.0)

        # =========== Phase C2: scores + one-hot (uses bhat from C1) ===========
        with tc.tile_pool(name="psC2", bufs=2, space="PSUM") as psC2:
            for mi in range(4):
                pss = psC2.tile([P, NH], F32, tag="pss")
                nc.tensor.matmul(out=pss[:], lhsT=metricT[0:HD, mi * P:(mi + 1) * P],
                                 rhs=bhat[0:HD, :], start=True, stop=True)
                mx = stat.tile([P, 8], F32, tag="mx")
                nc.vector.max(out=mx[:], in_=pss[:])
                nc.vector.tensor_scalar(out=onehot[:, mi, :], in0=pss[:],
                                        scalar1=mx[:, 0:1], scalar2=None,
                                        op0=OP.is_equal)

        with (
            tc.tile_pool(name="expp", bufs=6) as expp,
            tc.tile_pool(name="denp", bufs=2) as denp,
            tc.tile_pool(name="psST", bufs=2, space="PSUM") as psST,
            tc.tile_pool(name="psPV", bufs=2, space="PSUM") as psPV,
        ):
            for pr in range(6):
                qTt = qkT[pr]
                kTt = qkT[6 + pr]
                pvA = psPV.tile([P, N_TOK], F32, tag="pv")
                pvB = psPV.tile([P, N_TOK], F32, tag="pv")
                for jt in range(NT):
                    stA = psST.tile([P, N_TOK], F32, tag="st")
                    stB = psST.tile([P, N_TOK], F32, tag="st")
                    for ch in range(2):
                        sl = slice(ch * 512, (ch + 1) * 512)
                        nc.tensor.matmul(out=stA[:, sl],
                                         lhsT=kTt[0:64, jt * P:(jt + 1) * P],
                                         rhs=qTt[0:64, sl], start=True, stop=True,
                                         tile_position=(0, 0))
                        nc.tensor.matmul(out=stB[:, sl],
                                         lhsT=kTt[64:128, jt * P:(jt + 1) * P],
                                         rhs=qTt[64:128, sl], start=True, stop=True,
                                         tile_position=(64, 0))
                    eA = expp.tile([P, N_TOK], BF16, tag="e")
                    eB = expp.tile([P, N_TOK], BF16, tag="e")
                    nc.scalar.activation(out=eA[:], in_=stA[:], func=AF.Exp,
                                         bias=lsz_sb[:, jt:jt + 1], scale=0.125)
                    nc.scalar.activation(out=eB[:], in_=stB[:], func=AF.Exp,
                                         bias=lsz_sb[:, jt:jt + 1], scale=0.125)
                    for ch in range(2):
                        sl = slice(ch * 512, (ch + 1) * 512)
                        nc.tensor.matmul(out=pvA[0:HD + 1, sl],
                                         lhsT=vaug[jt][:, 2 * pr, :], rhs=eA[:, sl],
                                         start=(jt == 0), stop=(jt == NT - 1))
                        nc.tensor.matmul(out=pvB[0:HD + 1, sl],
                                         lhsT=vaug[jt][:, 2 * pr + 1, :], rhs=eB[:, sl],
                                         start=(jt == 0), stop=(jt == NT - 1))
                # stage to SBUF (frees PSUM fast), then divide by denominator row
                for hh, pv in ((0, pvA), (1, pvB)):
                    h = 2 * pr + hh
                    xa32 = denp.tile([P, N_TOK], F32, tag="xa32")
                    nc.vector.tensor_copy(out=xa32[0:HD + 1, :], in_=pv[0:HD + 1, :])
                    nc.sync.dma_start(out=den_scr[h:h + 1, :], in_=xa32[HD:HD + 1, :])
                    db = denp.tile([P, N_TOK], F32, tag="db")
                    _bcast_row(nc, db[0:HD, :], den_scr[h:h + 1, :], HD)
                    nc.vector.reciprocal(out=db[0:HD, :], in_=db[0:HD, :])
                    nc.vector.tensor_tensor(
                        out=xaT16[pr][hh * HD:(hh + 1) * HD, :],
                        in0=xa32[0:HD, :], in1=db[0:HD, :], op=OP.mult)
        cm_zt16.__exit__(None, None, None)
        cm_vaug.__exit__(None, None, None)
        cm_qkT.__exit__(None, None, None)

        # =========== Phase E: proj + residual -> x1w ===========
        with (
            tc.tile_pool(name="wE", bufs=1) as wE,
            tc.tile_pool(name="psE", bufs=4, space="PSUM") as psE,
        ):
            pj_sb = wE.tile([P, KC, C], BF16)
            wengs2 = (nc.sync,)
            for kc in range(KC):
                wengs2[kc % 1].dma_start(out=pj_sb[:, kc, :],
                                         in_=projT[kc * P:(kc + 1) * P, :])
            for t in range(NT):
                ps = psE.tile([P, N_TOK], F32)
                for (c0, c1) in ((0, 512), (512, 768)):
                    for kc in range(KC):
                        nc.tensor.matmul(out=ps[:, c0:c1],
                                         lhsT=xaT16[kc][:, t * P:(t + 1) * P],
                                         rhs=pj_sb[:, kc, c0:c1],
                                         start=(kc == 0), stop=(kc == KC - 1))
                if unit_size:
                    nc.vector.tensor_tensor(out=x1w[:, t, 0:C], in0=ps[:, 0:C],
                                            in1=xs[:, t, :], op=OP.add)
                    nc.gpsimd.tensor_copy(out=x1w[:, t, C:C + 1],
                                          in_=asz_sb[:, t:t + 1])
                else:
                    tt = scratch.tile([P, C], F32, tag="tt")
                    nc.vector.tensor_tensor(out=tt[:], in0=ps[:, 0:C],
                                            in1=xs[:, t, :], op=OP.add)
                    nc.gpsimd.tensor_scalar_mul(out=x1w[:, t, 0:C], in0=tt[:],
                                                scalar1=asz_sb[:, t:t + 1])
                    nc.gpsimd.tensor_copy(out=x1w[:, t, C:C + 1],
                                          in_=asz_sb[:, t:t + 1])
        cm_xaT.__exit__(None, None, None)
        cm_xs.__exit__(None, None, None)

        # merged-token buffer + MLP weight pools (DMAs prefetch during merge/LN2)
        cm_x2 = tc.tile_pool(name="p_x2", bufs=1)
        x2 = cm_x2.__enter__().tile([P, 4, C], F32)
        cm_wG1 = tc.tile_pool(name="wG1", bufs=1)
        f1_sb = cm_wG1.__enter__().tile([P, KC, 4 * C], BF16)
        wengs3 = (nc.sync, nc.gpsimd)
        for kc in range(KC):
            wengs3[kc % 2].dma_start(out=f1_sb[:, kc, :],
                                     in_=fc1T[kc * P:(kc + 1) * P, :])
        cm_wG2 = tc.tile_pool(name="wG2", bufs=1)
        f2_sb = cm_wG2.__enter__().tile([P, 24, C], BF16)
        for kc in range(24):
            wengs3[kc % 2].dma_start(out=f2_sb[:, kc, :],
                                     in_=fc2T[kc * P:(kc + 1) * P, :])

        # =========== Phase F: ToMe merge -> x2, out_sz ===========
        with tc.tile_pool(name="psF", bufs=2, space="PSUM") as psF:
            for mj in range(4):
                ps = psF.tile([P, N_TOK], F32)
                for (c0, c1) in ((0, 512), (512, C + 1)):
                    for kt in range(4):
                        nc.tensor.matmul(out=ps[:, c0:c1],
                                         lhsT=onehot[:, kt, mj * P:(mj + 1) * P],
                                         rhs=x1w[:, kt, c0:c1],
                                         start=(kt == 0), stop=(kt == 3))
                szt = stat.tile([P, 1], F32, tag="szt")
                nc.vector.tensor_scalar_add(out=szt[:], in0=ps[:, C:C + 1],
                                            scalar1=asz_sb[:, 4 + mj:5 + mj])
                nc.sync.dma_start(out=out_sz[mj * P:(mj + 1) * P, :], in_=szt[:])
                rsz = stat.tile([P, 1], F32, tag="rsz")
                nc.vector.reciprocal(out=rsz[:], in_=szt[:])
                num = scratch.tile([P, C], F32, tag="num")
                nc.vector.tensor_tensor(out=num[:], in0=ps[:, 0:C],
                                        in1=x1w[:, 4 + mj, 0:C], op=OP.add)
                nc.vector.tensor_scalar_mul(out=x2[:, mj, :], in0=num[:],
                                            scalar1=rsz[:])
        cm_x1w.__exit__(None, None, None)
        cm_oh.__exit__(None, None, None)

        # =========== Phase G: LN2 -> fc1+gelu -> fc2 + residual ===========
        cm_z2t = tc.tile_pool(name="p_z2t", bufs=1)
        z2t16 = cm_z2t.__enter__().tile([P, KC, NH], BF16)
        cm_h2g = tc.tile_pool(name="p_h2g", bufs=1)
        h2g = cm_h2g.__enter__().tile([P, 24, NH], BF16)

        with tc.tile_pool(name="psG", bufs=4, space="PSUM") as psG:
            for mj in range(4):
                z2 = scratch.tile([P, C], F32, tag="z")
                _layernorm_tile(nc, stat, x2[:, mj, :], z2[:], eps_t)
                # LN2 has consumed x2[mj]; fold the fc2 bias into it in place
                # (idle GPSIMD) so the output tail needs a single DVE add
                nc.gpsimd.tensor_tensor(out=x2[:, mj, :], in0=x2[:, mj, :],
                                        in1=f2b_b[:], op=OP.add)
                pt = psG.tile([P, C], F32, tag="tr", bufs=2)
                for kc in range(KC):
                    nc.tensor.transpose(out=pt[:, kc * P:(kc + 1) * P],
                                        in_=z2[:, kc * P:(kc + 1) * P],
                                        identity=ident[:])
                nc.scalar.copy(
                    out=z2t16[:, :, mj * P:(mj + 1) * P],
                    in_=pt[:].rearrange("p (a b) -> p a b", a=KC))

            for mo in range(24):
                ps = psG.tile([P, NH], F32, tag="f1")
                for kc in range(KC):
                    nc.tensor.matmul(out=ps[:],
                                     lhsT=f1_sb[:, kc, mo * P:(mo + 1) * P],
                                     rhs=z2t16[:, kc, :],
                                     start=(kc == 0), stop=(kc == KC - 1))
                nc.scalar.activation(out=h2g[:, mo, :], in_=ps[:], func=AF.Gelu,
                                     bias=b3_sb[:, mo:mo + 1], scale=1.0)

        with tc.tile_pool(name="psG2", bufs=3, space="PSUM") as psG2:
            for mj in range(4):
                ps = psG2.tile([P, N_TOK], F32)
                for (c0, c1) in ((0, 512), (512, 768)):
                    for kc in range(24):
                        nc.tensor.matmul(out=ps[:, c0:c1],
                                         lhsT=h2g[:, kc, mj * P:(mj + 1) * P],
                                         rhs=f2_sb[:, kc, c0:c1],
                                         start=(kc == 0), stop=(kc == 23))
                x3 = scratch.tile([P, C], F32, tag="num")
                nc.vector.tensor_tensor(out=x3[:], in0=ps[:, 0:C], in1=x2[:, mj, :],
                                        op=OP.add)
                nc.sync.dma_start(out=out_x[mj * P:(mj + 1) * P, :], in_=x3[:])

        cm_h2g.__exit__(None, None, None)
        cm_z2t.__exit__(None, None, None)
        cm_wG2.__exit__(None, None, None)
        cm_wG1.__exit__(None, None, None)
        cm_x2.__exit__(None, None, None)
        cm_stat.__exit__(None, None, None)
        cm_scr.__exit__(None, None, None)
        cm_const.__exit__(None, None, None)

    return nc


_NC = {}


def _get_nc(unit_size):
    if unit_size not in _NC:
        _NC[unit_size] = _build_nc(unit_size)
    return _NC[unit_size]


def kernel(x, attn_size, ln1_w, ln1_b, qkv_w, proj_w, proj_b,
           ln2_w, ln2_b, fc1_w, fc1_b, fc2_w, fc2_b):
    from concourse.bass_utils import run_bass_kernel_spmd

    f32 = np.float32
    bf16 = ml_dtypes.bfloat16
    x = np.asarray(x, f32)
    attn_size = np.asarray(attn_size, f32)
    B = x.shape[0]
    assert B == N_CORES

    # fold LN1 affine into qkv weights
    W1 = np.asarray(qkv_w, f32) * np.asarray(ln1_w, f32)[None, :]
    b1 = np.asarray(qkv_w, f32) @ np.asarray(ln1_b, f32)
    Wq, Wk, Wv = W1[0:C], W1[C:2 * C], W1[2 * C:3 * C]
    bq, bk, bv = b1[0:C], b1[C:2 * C], b1[2 * C:3 * C]
    wqkT_h = np.ascontiguousarray(np.concatenate([Wq, Wk], 0).T).astype(bf16)
    wvT_h = np.ascontiguousarray(Wv.T).astype(bf16)
    bqk_h = np.ascontiguousarray(
        np.concatenate([bq, bk], 0).reshape(12, P).T).astype(f32)
    Wm = Wk.reshape(H, HD, C).mean(0)
    bm_h = bk.reshape(H, HD).mean(0).reshape(HD, 1).astype(f32)
    wmT_h = np.ascontiguousarray(Wm.T).astype(f32)
    # fold LN2 affine into fc1
    W3 = np.asarray(fc1_w, f32) * np.asarray(ln2_w, f32)[None, :]
    b3v = np.asarray(fc1_b, f32) + np.asarray(fc1_w, f32) @ np.asarray(ln2_b, f32)
    fc1T_h = np.ascontiguousarray(W3.T).astype(bf16)
    b3_h = np.ascontiguousarray(b3v.reshape(24, P).T).astype(f32)
    projT_h = np.ascontiguousarray(np.asarray(proj_w, f32).T).astype(bf16)
    fc2T_h = np.ascontiguousarray(np.asarray(fc2_w, f32).T).astype(bf16)

    perm = np.concatenate([np.arange(0, N_TOK, 2), np.arange(1, N_TOK, 2)])
    wmT2_h = np.ascontiguousarray(
        wmT_h.reshape(KC, P, HD).transpose(1, 0, 2).reshape(P, KC * HD))
    rpack_h = np.concatenate(
        [bv, np.asarray(proj_b, f32), np.asarray(fc2_b, f32)]).reshape(1, 3 * C)
    rpack_h = np.ascontiguousarray(rpack_h).astype(f32)
    shared = {
        "wqkT": wqkT_h, "wvT": wvT_h, "wmT2": wmT2_h,
        "projT": projT_h, "fc1T": fc1T_h, "fc2T": fc2T_h,
        "rpack": rpack_h,
    }
    in_maps = []
    for b in range(B):
        xb = np.ascontiguousarray(x[b][perm])
        ab = np.ascontiguousarray(attn_size[b][perm]).astype(f32)
        lb = np.log(ab).astype(f32)
        cpack_h = np.concatenate([
            bqk_h, b3_h, np.pad(bm_h.reshape(HD), (0, P - HD)).reshape(P, 1),
            ab.reshape(NT, P).T, lb.reshape(NT, P).T,
        ], axis=1).astype(f32)
        in_maps.append({
            "xp": xb, "cpack": np.ascontiguousarray(cpack_h),
            **shared,
        })

    unit_size = bool(np.all(attn_size == 1.0))
    nc = _get_nc(unit_size)
    import os
    trace = bool(os.environ.get("TOME_TRACE"))
    res = run_bass_kernel_spmd(nc, in_maps, core_ids=list(range(N_CORES)),
                               trace=trace)
    if trace and res.exec_time_ns is not None:
        print(f"HW exec time: {res.exec_time_ns} ns")
        if res.instructions_and_trace is not None:
            print(f"trace: {res.instructions_and_trace[1]}")
    out = np.stack([r["out_x"] for r in res.results]).astype(f32)
    size = np.stack([r["out_sz"] for r in res.results]).astype(f32)
    return out, size
